# revision 31
# baseline (speedup 1.0000x reference)
"""AdaIN statistics kernel for TRN2, SPMD across 8 NeuronCores.

Input : f_vol [32, 512, 64, 64] f32
Output: [32, 1024] f32 = concat([mean over (h,w), unbiased std over (h,w)], axis=-1)

Sharding: data-parallel over batch — each of the 8 cores handles 4 batches
([4, 512, 64, 64] shard, 32 MiB). No collectives; the host concatenates the
8 per-core [4, 1024] outputs.

Per core: view the shard as 2048 rows (b*512+c) x 4096 spatial elems.
The shard is streamed in SLABS: a slab with m rows/partition loads
128*m consecutive rows, partition p holding rows base+p*m .. +m (so each
partition's DRAM chunk is m*16 KiB contiguous -> m*16 KiB DMA descriptors;
per-SDMA-engine rate is descriptor-size-bound). m=2 slabs stream at the
HBM cap; m=1 slabs at the end keep the compute tail short.

Raw Bass with manual semaphores (Tile's scheduler emits 2 sync-waits on
slot-reuse DMAs, which this compiler's static-DMA encoding cannot hold):
  SP  : input slab DMAs (ring of 6 x 32 KiB/partition SBUF slots), final
        output DMA
  DVE : 8 bn_stats per row + bn_aggr per row -> (mean, biased var); final
        32x32 block transposes of the stat matrix
  ACT : mean copy + sqrt(var * N/(N-1)) per row into F[128, 32]; one slab
        is consumed by ACT itself (Copy/Square+accumulate) to offload DVE

All stats collect into F[p, col] (col = b*8 + q*4 + cb). At the end DVE
transposes F -> T[32, 128] so the single output DMA writes 32 contiguous
512 B runs (per-partition 4 B descriptors would hit DRAM read-modify-write
and cost ~7 us of completion latency).

DMA completion is not FIFO across in-flight transfers, so each slab gets
its own single-use DMA-completion semaphore. Every cross-instruction data
edge is covered by an explicit semaphore observation so the CoreSim race
detector can verify the design.
"""

from contextlib import ExitStack

import numpy as np

B, C, H, W = 32, 512, 64, 64
N_CORES = 8
B_LOCAL = B // N_CORES  # 4
N = H * W  # 4096
P = 128
ROWS = B_LOCAL * C  # 2048
G = N // 512  # bn_stats groups per row = 8
NCOL = 2 * ROWS // P  # 32 stat columns (b, q, cb)

# rows-per-partition per slab; each slab (128*m rows) must stay inside one
# batch. Consumer: 'dve' = bn_stats path, 'act' = ScalarE accumulate path.
SLABS = [2, 2, 2, 2, 2, 2, 2, 1, 1]
CONSUMER = ["dve"] * 7 + ["act", "dve"]
NBUF = 6  # input slab ring slots (NBUF x MMAX*16 KiB/partition)
NSMALL = 4  # stats/mv ring slots
HOIST = 1  # emit an ACT slab's accumulate before the epilogue of slab j-HOIST
# Split the final DVE slab's columns: part A (TAILSPLIT groups) is issued
# early so only part B (8-TAILSPLIT groups) trails the stream end.
# Measured as a ~2 us regression (the 12 KiB-descriptor A part slows the
# stream more than the shorter tail gains) — keep disabled.
TAILSPLIT = 0

_CACHE = {}


def _build():
    import concourse.bass as bass
    from concourse import mybir

    nc = bass.Bass()
    x_ext = nc.declare_dram_parameter(
        "f_vol", [B_LOCAL, C, H, W], mybir.dt.float32, isOutput=False
    )
    out_ext = nc.declare_dram_parameter(
        "out", [B_LOCAL, 2 * C], mybir.dt.float32, isOutput=True
    )

    x = x_ext.ap().rearrange("b c h w -> (b c) (h w)")  # [2048, 4096]

    assert sum(SLABS) * P == ROWS and len(CONSUMER) == len(SLABS)
    MMAX = max(SLABS)
    nslabs = len(SLABS)
    base_rows = [P * sum(SLABS[:j]) for j in range(nslabs)]
    for j, m in enumerate(SLABS):
        assert (base_rows[j] % C) + P * m <= C, f"slab {j} crosses a batch"

    # --- plan: cumulative semaphore targets per slab ---
    # dve_stats: +1 per bn_stats (DVE slabs);  act_stats: +1 per ACT
    # accumulate pass (2 per row, ACT slabs). mv_ready: +1 per bn_aggr.
    # act_done: +2 per DVE-slab row (mean copy + sqrt) or +4 per ACT-slab
    # row (epilogue: mean, square, negate, sqrt) — all on ACT.
    dve_after, act_stats_after, mv_after, actd_after = [], [], [], []
    cd = ca = cm = cact = 0
    for j, m in enumerate(SLABS):
        if CONSUMER[j] == "dve":
            cd += G * m
            cm += m
            cact += 2 * m
        else:
            ca += 2 * m
            cact += 4 * m
        dve_after.append(cd)
        act_stats_after.append(ca)
        mv_after.append(cm)
        actd_after.append(cact)
    ACT_TOTAL = cact

    # m==1 tail slabs route through the F/T transpose path; m>=2 slabs keep
    # direct per-slab out-DMAs (their completion latency hides mid-stream)
    TAIL = [j for j, m in enumerate(SLABS) if m == 1]
    assert TAIL == list(range(nslabs - len(TAIL), nslabs)), "m=1 slabs must be last"
    tb, tc0 = divmod(base_rows[TAIL[0]], C)
    for ti, j in enumerate(TAIL):
        bj, c0j = divmod(base_rows[j], C)
        assert bj == tb and c0j == tc0 + ti * P, "tail slabs must be consecutive"

    with ExitStack() as ctx:
        block = ctx.enter_context(nc.Block(no_gpsimd_drain=True))
        dma_in = [
            ctx.enter_context(nc.semaphore(f"dma_in{j}")) for j in range(nslabs)
        ]
        dma_out = [
            ctx.enter_context(nc.semaphore(f"dma_out{s}")) for s in range(NSMALL)
        ]
        dma_fin = ctx.enter_context(nc.semaphore("dma_fin"))
        dma_inB = ctx.enter_context(nc.semaphore("dma_inB"))
        dve_stats = ctx.enter_context(nc.semaphore("dve_stats"))
        act_stats = ctx.enter_context(nc.semaphore("act_stats"))
        mv_ready = ctx.enter_context(nc.semaphore("mv_ready"))
        act_done = ctx.enter_context(nc.semaphore("act_done"))
        trans_done = ctx.enter_context(nc.semaphore("trans_done"))
        xt = ctx.enter_context(
            nc.sbuf_tensor("xt", [P, NBUF, MMAX * N], mybir.dt.float32)
        )
        stats = ctx.enter_context(
            nc.sbuf_tensor("stats", [P, NSMALL, MMAX, G, 6], mybir.dt.float32)
        )
        mv = ctx.enter_context(
            nc.sbuf_tensor("mv", [P, NSMALL, MMAX, 2], mybir.dt.float32)
        )
        res = ctx.enter_context(
            nc.sbuf_tensor("res", [P, NSMALL, 2, MMAX], mybir.dt.float32)
        )
        F = ctx.enter_context(nc.sbuf_tensor("F", [P, 32], mybir.dt.float32))
        T = ctx.enter_context(nc.sbuf_tensor("T", [P, P], mybir.dt.float32))
        # ACT-slab accumulators: [sum, sumsq, tmp] per row, no reuse
        acc = ctx.enter_context(
            nc.sbuf_tensor("acc", [P, MMAX, 3], mybir.dt.float32)
        )

        # per-slab direct out-DMAs for m>=2 slabs only
        out_total = {s: 0 for s in range(NSMALL)}
        out_after = [0] * nslabs  # dma_out[j % NSMALL] value after slab j
        last_use = {}  # slot -> value to wait for before reuse
        wait_before = [0] * nslabs
        for j, m in enumerate(SLABS):
            s = j % NSMALL
            if m >= 2:
                wait_before[j] = last_use.get(s, 0)
                out_total[s] += 16
                out_after[j] = out_total[s]
                last_use[s] = out_total[s]

        def slot_free_waits(eng, j):
            """Waits before rewriting xt slot (j % NBUF) for slab j."""
            if j < NBUF:
                return
            jp = j - NBUF
            if CONSUMER[jp] == "dve":
                eng.wait_ge(dve_stats, dve_after[jp])
            else:
                eng.wait_ge(act_stats, act_stats_after[jp])
            eng.wait_ge(dma_in[jp], 16)

        # issue order: the split slab's A part goes out one position early
        jsplit = nslabs - 1 if TAILSPLIT and SLABS[-1] == 1 and CONSUMER[-1] == "dve" else None
        issue_order = []
        for j in range(nslabs):
            if j == jsplit:
                continue
            if jsplit is not None and j == nslabs - 2:
                issue_order.append((jsplit, "A"))
            issue_order.append((j, None))
        if jsplit is not None:
            issue_order.append((jsplit, "B"))

        @block.sync
        def _(sync):
            for j, part in issue_order:
                m = SLABS[j]
                if part is None:
                    slot_free_waits(sync, j)
                    src = x[base_rows[j] : base_rows[j] + P * m, :].rearrange(
                        "(p m) f -> p (m f)", m=m
                    )
                    sync.dma_start(
                        out=xt[:, j % NBUF, 0 : m * N], in_=src
                    ).then_inc(dma_in[j], 16)
                elif part == "A":
                    slot_free_waits(sync, j)
                    cA = 512 * TAILSPLIT
                    srcA = bass.AP(
                        tensor=x_ext,
                        offset=base_rows[j] * N,
                        ap=[[N, P], [1, cA]],
                    )
                    sync.dma_start(
                        out=xt[:, j % NBUF, 0:cA], in_=srcA
                    ).then_inc(dma_in[j], 16)
                else:
                    # B: same slot, disjoint region; sync already observed the
                    # slot-free sems when issuing A
                    cA = 512 * TAILSPLIT
                    srcB = bass.AP(
                        tensor=x_ext,
                        offset=base_rows[j] * N + cA,
                        ap=[[N, P], [1, N - cA]],
                    )
                    sync.dma_start(
                        out=xt[:, j % NBUF, cA:N], in_=srcB
                    ).then_inc(dma_inB, 16)
            # tail stats: T rows (2*ti+q) -> contiguous 512 B runs
            sync.wait_ge(trans_done, 1 + P // 32)
            nt = len(TAIL)
            dst = bass.AP(
                tensor=out_ext,
                offset=tb * 2 * C + tc0,
                ap=[[P, nt], [C, 2], [1, P]],
            )
            sync.dma_start(out=dst, in_=T[0 : 2 * nt, 0:P]).then_inc(dma_fin, 16)
            for s in range(NSMALL):
                if out_total[s]:
                    sync.wait_ge(dma_out[s], out_total[s])
            sync.wait_ge(dma_fin, 16)

        @block.vector
        def _(vector):
            # F cols beyond the tail stats stay zero; transpose reads all 32
            vector.memset(F[:, :], 0.0).then_inc(trans_done, 1)
            prev_dve = [jj for jj in range(nslabs) if CONSUMER[jj] == "dve"]
            for j, m in enumerate(SLABS):
                if CONSUMER[j] != "dve":
                    continue
                k = j % NBUF
                s = j % NSMALL
                vector.wait_ge(dma_in[j], 16)
                # stats/mv slot WAR vs the previous DVE slab that used slot s
                pi = prev_dve.index(j)
                jp = None
                for jj in prev_dve[:pi][::-1]:
                    if jj % NSMALL == s:
                        jp = jj
                        break
                if jp is not None:
                    vector.wait_ge(mv_ready, mv_after[jp])
                split = j == nslabs - 1 and m == 1 and TAILSPLIT
                for r in range(m):
                    for g in range(G):
                        if split and g == TAILSPLIT:
                            vector.wait_ge(dma_inB, 16)
                        vector.bn_stats(
                            out=stats[:, s, r, g, :],
                            in_=xt[:, k, (r * G + g) * 512 : (r * G + g + 1) * 512],
                        ).then_inc(dve_stats, 1)
                if jp is not None:
                    vector.wait_ge(act_done, actd_after[jp])
                # stats RAW: this slab's bn_stats writes retired
                vector.wait_ge(dve_stats, dve_after[j])
                for r in range(m):
                    vector.bn_aggr(
                        out=mv[:, s, r, :], in_=stats[:, s, r, :, :]
                    ).then_inc(mv_ready, 1)
            # all F columns written -> 32x32 block transposes of F into T:
            # T[c, 32*blk + pp] = F[32*blk + pp, c]
            vector.wait_ge(act_done, ACT_TOTAL)
            vector.wait_ge(trans_done, 1)  # observe the F memset (same engine)
            for blk in range(P // 32):
                vector.transpose(
                    out=T[0:32, blk * 32 : blk * 32 + 32],
                    in_=F[blk * 32 : blk * 32 + 32, 0:32],
                ).then_inc(trans_done, 1)

        @block.scalar
        def _(scalar):
            A = 1.0 / np.sqrt(float(N) * (N - 1))

            def act_accumulate(j, m, k):
                # sum (Copy+accum) then sumsq (Square+accum), in-place on xt
                scalar.wait_ge(dma_in[j], 16)
                base_as = act_stats_after[j] - 2 * m
                for r in range(m):
                    row = xt[:, k, r * N : (r + 1) * N]
                    scalar.activation(
                        out=row,
                        in_=row,
                        func=mybir.ActivationFunctionType.Copy,
                        accum_out=acc[:, r, 0:1],
                    ).then_inc(act_stats, 1)
                    # observe the Copy (xt write + acc[0]) before Square
                    scalar.wait_ge(act_stats, base_as + 2 * r + 1)
                    scalar.activation(
                        out=row,
                        in_=row,
                        func=mybir.ActivationFunctionType.Square,
                        accum_out=acc[:, r, 1:2],
                    ).then_inc(act_stats, 1)

            def stat_dsts(j, m, s):
                """Per-row (mean_dst, std_dst) APs for this slab's stats."""
                if m == 1:
                    scalar.wait_ge(trans_done, 1)  # F memset observed
                    ti = TAIL.index(j)
                    return [(F[:, 2 * ti : 2 * ti + 1], F[:, 2 * ti + 1 : 2 * ti + 2])]
                return [
                    (res[:, s, 0, r : r + 1], res[:, s, 1, r : r + 1])
                    for r in range(m)
                ]

            def finish_direct(j, m, s):
                """res RAW wait + direct out-DMA for m>=2 slabs."""
                b, c0 = divmod(base_rows[j], C)
                scalar.wait_ge(act_done, actd_after[j])
                dst = bass.AP(
                    tensor=out_ext,
                    offset=b * 2 * C + c0,
                    ap=[[m, P], [C, 2], [1, m]],
                )
                scalar.dma_start(out=dst, in_=res[:, s, :, 0:m]).then_inc(
                    dma_out[s], 16
                )

            def act_epilogue(j, m):
                s = j % NSMALL
                dsts = stat_dsts(j, m, s)
                scalar.wait_ge(act_stats, act_stats_after[j])
                if m >= 2 and wait_before[j]:
                    scalar.wait_ge(dma_out[j % NSMALL], wait_before[j])
                ad = actd_after[j] - 4 * m  # running act_done value
                for r in range(m):
                    mean_dst, std_dst = dsts[r]
                    # mean = sum / N
                    scalar.activation(
                        out=mean_dst,
                        in_=acc[:, r, 0:1],
                        func=mybir.ActivationFunctionType.Copy,
                        scale=1.0 / N,
                    ).then_inc(act_done, 1)
                    # tmp = (sum*A)^2 = sum^2/(N(N-1))
                    scalar.activation(
                        out=acc[:, r, 2:3],
                        in_=acc[:, r, 0:1],
                        func=mybir.ActivationFunctionType.Square,
                        scale=A,
                    ).then_inc(act_done, 1)
                    ad += 2
                    scalar.wait_ge(act_done, ad)
                    scalar.activation(
                        out=acc[:, r, 2:3],
                        in_=acc[:, r, 2:3],
                        func=mybir.ActivationFunctionType.Copy,
                        scale=-1.0,
                    ).then_inc(act_done, 1)
                    ad += 1
                    scalar.wait_ge(act_done, ad)
                    # std = sqrt(sumsq/(N-1) - sum^2/(N(N-1)))
                    scalar.activation(
                        out=std_dst,
                        in_=acc[:, r, 1:2],
                        func=mybir.ActivationFunctionType.Sqrt,
                        scale=1.0 / (N - 1),
                        bias=acc[:, r, 2:3],
                    ).then_inc(act_done, 1)
                    ad += 1
                if m >= 2:
                    finish_direct(j, m, s)

            def dve_epilogue(j, m):
                s = j % NSMALL
                dsts = stat_dsts(j, m, s)
                scalar.wait_ge(mv_ready, mv_after[j])
                if m >= 2 and wait_before[j]:
                    scalar.wait_ge(dma_out[j % NSMALL], wait_before[j])
                for r in range(m):
                    mean_dst, std_dst = dsts[r]
                    scalar.copy(out=mean_dst, in_=mv[:, s, r, 0:1]).then_inc(
                        act_done, 1
                    )
                    scalar.activation(
                        out=std_dst,
                        in_=mv[:, s, r, 1:2],
                        func=mybir.ActivationFunctionType.Sqrt,
                        scale=float(N) / (N - 1),
                    ).then_inc(act_done, 1)
                if m >= 2:
                    finish_direct(j, m, s)

            # Emission order: an ACT slab's accumulate is hoisted before the
            # previous slab's epilogue so it starts at DMA arrival instead of
            # queueing behind mv_ready stalls.
            emitted = set()
            for j, m in enumerate(SLABS):
                nj = j + HOIST
                if nj < nslabs and CONSUMER[nj] == "act" and nj not in emitted:
                    act_accumulate(nj, SLABS[nj], nj % NBUF)
                    emitted.add(nj)
                if CONSUMER[j] == "dve":
                    dve_epilogue(j, m)
                else:
                    if j not in emitted:
                        act_accumulate(j, m, j % NBUF)
                        emitted.add(j)
                    act_epilogue(j, m)

    return nc


def kernel(f_vol: np.ndarray) -> np.ndarray:
    from concourse.bass_utils import run_bass_kernel_spmd

    if "nc" not in _CACHE:
        _CACHE["nc"] = _build()
    nc = _CACHE["nc"]

    f_vol = np.ascontiguousarray(f_vol, dtype=np.float32)
    in_maps = [
        {"f_vol": f_vol[i * B_LOCAL : (i + 1) * B_LOCAL]} for i in range(N_CORES)
    ]
    res = run_bass_kernel_spmd(nc, in_maps, core_ids=list(range(N_CORES)))
    return np.concatenate([res.results[i]["out"] for i in range(N_CORES)], axis=0)


# revision 32
# speedup vs baseline: 1.1644x; 1.1644x over previous
"""AdaIN statistics kernel for TRN2, SPMD across 8 NeuronCores.

Input : f_vol [32, 512, 64, 64] f32
Output: [32, 1024] f32 = concat([mean over (h,w), unbiased std over (h,w)], axis=-1)

Sharding: data-parallel over batch — each of the 8 cores handles 4 batches
([4, 512, 64, 64] shard, 32 MiB). No collectives; the host concatenates the
8 per-core [4, 1024] outputs.

Per core: view the shard as 2048 rows (b*512+c) x 4096 spatial elems.
The shard is streamed in SLABS: a slab with m rows/partition loads
128*m consecutive rows, partition p holding rows base+p*m .. +m, so each
partition's DRAM chunk is m*16 KiB contiguous -> m*16 KiB DMA descriptors.
Per-SDMA-engine throughput is descriptor-size-bound (~16 GB/s at 16 KiB,
~26 GB/s at 32-64 KiB), so m=2 slabs stream at the HBM cap while two m=1
slabs at the end keep the compute tail short.

Raw Bass with manual semaphores (Tile's scheduler emits 2 sync-waits on
slot-reuse DMAs, which this compiler's static-DMA encoding cannot hold):
  SP  : input slab DMAs (ring of 6 x 32 KiB/partition SBUF slots) on one
        HWDGE ring; the single final output DMA
  DVE : 8x bn_stats + bn_aggr per row -> (mean, biased var); final 32x32
        stream-transposes of the tail-stat matrix F -> T
  ACT : per-row epilogue mean copy + sqrt(var * N/(N-1)); consumes the
        second-to-last slab itself via Copy/Square+accumulate passes so
        the last slab never queues behind DVE's backlog; issues the
        per-slab output DMAs for m>=2 slabs

Outputs: m=2 slabs DMA their [128, 2, m] stats directly (interleaved
channel layout, 3D AP; the ~7 us completion latency of the tiny
per-partition descriptors hides mid-stream). The two m=1 tail slabs write
stat columns of F[128, 4]; DVE block-transposes F -> T so the one final
DMA writes 4 contiguous 512 B runs — per-partition 4 B descriptors would
hit DRAM read-modify-write and cost ~7 us of exposed completion latency
(measured; the transpose path cuts it to ~1.1 us).

DMA completion is not FIFO across in-flight transfers, so each slab gets
its own single-use DMA-completion semaphore (SWDGE additionally requires
sem values to start at 0, so semaphores are never reused). Every
cross-instruction data edge is covered by an explicit semaphore
observation so the CoreSim race detector can verify the design.
"""

from contextlib import ExitStack

import numpy as np

B, C, H, W = 32, 512, 64, 64
N_CORES = 8
B_LOCAL = B // N_CORES  # 4
N = H * W  # 4096
P = 128
ROWS = B_LOCAL * C  # 2048
G = N // 512  # bn_stats groups per row = 8
NCOL = 2 * ROWS // P  # 32 stat columns (b, q, cb)

# rows-per-partition per slab; each slab (128*m rows) must stay inside one
# batch. Consumer: 'dve' = bn_stats path, 'act' = ScalarE accumulate path.
SLABS = [2, 2, 2, 2, 2, 2, 2, 1, 1]
CONSUMER = ["dve"] * 7 + ["act", "dve"]
NBUF = 6  # input slab ring slots (NBUF x MMAX*16 KiB/partition)
NSMALL = 4  # stats/mv ring slots
HOIST = 1  # emit an ACT slab's accumulate before the epilogue of slab j-HOIST
# Split the final DVE slab's columns: part A (TAILSPLIT groups) is issued
# early so only part B (8-TAILSPLIT groups) trails the stream end.
# Measured as a ~2 us regression (the 12 KiB-descriptor A part slows the
# stream more than the shorter tail gains) — keep disabled.
TAILSPLIT = 0

_CACHE = {}


def _build():
    import concourse.bass as bass
    from concourse import mybir

    nc = bass.Bass()
    x_ext = nc.declare_dram_parameter(
        "f_vol", [B_LOCAL, C, H, W], mybir.dt.float32, isOutput=False
    )
    out_ext = nc.declare_dram_parameter(
        "out", [B_LOCAL, 2 * C], mybir.dt.float32, isOutput=True
    )

    x = x_ext.ap().rearrange("b c h w -> (b c) (h w)")  # [2048, 4096]

    assert sum(SLABS) * P == ROWS and len(CONSUMER) == len(SLABS)
    MMAX = max(SLABS)
    nslabs = len(SLABS)
    base_rows = [P * sum(SLABS[:j]) for j in range(nslabs)]
    for j, m in enumerate(SLABS):
        assert (base_rows[j] % C) + P * m <= C, f"slab {j} crosses a batch"

    # --- plan: cumulative semaphore targets per slab ---
    # dve_stats: +1 per bn_stats (DVE slabs);  act_stats: +1 per ACT
    # accumulate pass (2 per row, ACT slabs). mv_ready: +1 per bn_aggr.
    # act_done: +2 per DVE-slab row (mean copy + sqrt) or +4 per ACT-slab
    # row (epilogue: mean, square, negate, sqrt) — all on ACT.
    dve_after, act_stats_after, mv_after, actd_after = [], [], [], []
    cd = ca = cm = cact = 0
    for j, m in enumerate(SLABS):
        if CONSUMER[j] == "dve":
            cd += G * m
            cm += m
            cact += 2 * m
        else:
            ca += 2 * m
            cact += 4 * m
        dve_after.append(cd)
        act_stats_after.append(ca)
        mv_after.append(cm)
        actd_after.append(cact)
    ACT_TOTAL = cact

    # m==1 tail slabs route through the F/T transpose path; m>=2 slabs keep
    # direct per-slab out-DMAs (their completion latency hides mid-stream)
    TAIL = [j for j, m in enumerate(SLABS) if m == 1]
    assert TAIL == list(range(nslabs - len(TAIL), nslabs)), "m=1 slabs must be last"
    tb, tc0 = divmod(base_rows[TAIL[0]], C)
    for ti, j in enumerate(TAIL):
        bj, c0j = divmod(base_rows[j], C)
        assert bj == tb and c0j == tc0 + ti * P, "tail slabs must be consecutive"

    with ExitStack() as ctx:
        block = ctx.enter_context(nc.Block(no_gpsimd_drain=True))
        dma_in = [
            ctx.enter_context(nc.semaphore(f"dma_in{j}")) for j in range(nslabs)
        ]
        dma_out = [
            ctx.enter_context(nc.semaphore(f"dma_out{s}")) for s in range(NSMALL)
        ]
        dma_fin = ctx.enter_context(nc.semaphore("dma_fin"))
        dma_inB = ctx.enter_context(nc.semaphore("dma_inB"))
        dve_stats = ctx.enter_context(nc.semaphore("dve_stats"))
        act_stats = ctx.enter_context(nc.semaphore("act_stats"))
        mv_ready = ctx.enter_context(nc.semaphore("mv_ready"))
        act_done = ctx.enter_context(nc.semaphore("act_done"))
        trans_done = ctx.enter_context(nc.semaphore("trans_done"))
        xt = ctx.enter_context(
            nc.sbuf_tensor("xt", [P, NBUF, MMAX * N], mybir.dt.float32)
        )
        stats = ctx.enter_context(
            nc.sbuf_tensor("stats", [P, NSMALL, MMAX, G, 6], mybir.dt.float32)
        )
        mv = ctx.enter_context(
            nc.sbuf_tensor("mv", [P, NSMALL, MMAX, 2], mybir.dt.float32)
        )
        res = ctx.enter_context(
            nc.sbuf_tensor("res", [P, NSMALL, 2, MMAX], mybir.dt.float32)
        )
        F = ctx.enter_context(nc.sbuf_tensor("F", [P, 32], mybir.dt.float32))
        T = ctx.enter_context(nc.sbuf_tensor("T", [P, P], mybir.dt.float32))
        # ACT-slab accumulators: [sum, sumsq, tmp] per row, no reuse
        acc = ctx.enter_context(
            nc.sbuf_tensor("acc", [P, MMAX, 3], mybir.dt.float32)
        )

        # per-slab direct out-DMAs for m>=2 slabs only
        out_total = {s: 0 for s in range(NSMALL)}
        out_after = [0] * nslabs  # dma_out[j % NSMALL] value after slab j
        last_use = {}  # slot -> value to wait for before reuse
        wait_before = [0] * nslabs
        for j, m in enumerate(SLABS):
            s = j % NSMALL
            if m >= 2:
                wait_before[j] = last_use.get(s, 0)
                out_total[s] += 16
                out_after[j] = out_total[s]
                last_use[s] = out_total[s]

        def slot_free_waits(eng, j):
            """Waits before rewriting xt slot (j % NBUF) for slab j."""
            if j < NBUF:
                return
            jp = j - NBUF
            if CONSUMER[jp] == "dve":
                eng.wait_ge(dve_stats, dve_after[jp])
            else:
                eng.wait_ge(act_stats, act_stats_after[jp])
            eng.wait_ge(dma_in[jp], 16)

        # issue order: the split slab's A part goes out one position early
        jsplit = nslabs - 1 if TAILSPLIT and SLABS[-1] == 1 and CONSUMER[-1] == "dve" else None
        issue_order = []
        for j in range(nslabs):
            if j == jsplit:
                continue
            if jsplit is not None and j == nslabs - 2:
                issue_order.append((jsplit, "A"))
            issue_order.append((j, None))
        if jsplit is not None:
            issue_order.append((jsplit, "B"))

        @block.sync
        def _(sync):
            for j, part in issue_order:
                m = SLABS[j]
                if part is None:
                    slot_free_waits(sync, j)
                    src = x[base_rows[j] : base_rows[j] + P * m, :].rearrange(
                        "(p m) f -> p (m f)", m=m
                    )
                    sync.dma_start(
                        out=xt[:, j % NBUF, 0 : m * N], in_=src
                    ).then_inc(dma_in[j], 16)
                elif part == "A":
                    slot_free_waits(sync, j)
                    cA = 512 * TAILSPLIT
                    srcA = bass.AP(
                        tensor=x_ext,
                        offset=base_rows[j] * N,
                        ap=[[N, P], [1, cA]],
                    )
                    sync.dma_start(
                        out=xt[:, j % NBUF, 0:cA], in_=srcA
                    ).then_inc(dma_in[j], 16)
                else:
                    # B: same slot, disjoint region; sync already observed the
                    # slot-free sems when issuing A
                    cA = 512 * TAILSPLIT
                    srcB = bass.AP(
                        tensor=x_ext,
                        offset=base_rows[j] * N + cA,
                        ap=[[N, P], [1, N - cA]],
                    )
                    sync.dma_start(
                        out=xt[:, j % NBUF, cA:N], in_=srcB
                    ).then_inc(dma_inB, 16)
            # tail stats: T rows (2*ti+q) -> contiguous 512 B runs
            sync.wait_ge(trans_done, 1 + P // 32)
            nt = len(TAIL)
            dst = bass.AP(
                tensor=out_ext,
                offset=tb * 2 * C + tc0,
                ap=[[P, nt], [C, 2], [1, P]],
            )
            sync.dma_start(out=dst, in_=T[0 : 2 * nt, 0:P]).then_inc(dma_fin, 16)
            for s in range(NSMALL):
                if out_total[s]:
                    sync.wait_ge(dma_out[s], out_total[s])
            sync.wait_ge(dma_fin, 16)

        @block.vector
        def _(vector):
            # F cols beyond the tail stats stay zero; transpose reads all 32
            vector.memset(F[:, :], 0.0).then_inc(trans_done, 1)
            prev_dve = [jj for jj in range(nslabs) if CONSUMER[jj] == "dve"]
            for j, m in enumerate(SLABS):
                if CONSUMER[j] != "dve":
                    continue
                k = j % NBUF
                s = j % NSMALL
                vector.wait_ge(dma_in[j], 16)
                # stats/mv slot WAR vs the previous DVE slab that used slot s
                pi = prev_dve.index(j)
                jp = None
                for jj in prev_dve[:pi][::-1]:
                    if jj % NSMALL == s:
                        jp = jj
                        break
                if jp is not None:
                    vector.wait_ge(mv_ready, mv_after[jp])
                split = j == nslabs - 1 and m == 1 and TAILSPLIT
                for r in range(m):
                    for g in range(G):
                        if split and g == TAILSPLIT:
                            vector.wait_ge(dma_inB, 16)
                        vector.bn_stats(
                            out=stats[:, s, r, g, :],
                            in_=xt[:, k, (r * G + g) * 512 : (r * G + g + 1) * 512],
                        ).then_inc(dve_stats, 1)
                if jp is not None:
                    vector.wait_ge(act_done, actd_after[jp])
                # stats RAW: this slab's bn_stats writes retired
                vector.wait_ge(dve_stats, dve_after[j])
                for r in range(m):
                    vector.bn_aggr(
                        out=mv[:, s, r, :], in_=stats[:, s, r, :, :]
                    ).then_inc(mv_ready, 1)
            # all F columns written -> 32x32 block transposes of F into T:
            # T[c, 32*blk + pp] = F[32*blk + pp, c]
            vector.wait_ge(act_done, ACT_TOTAL)
            vector.wait_ge(trans_done, 1)  # observe the F memset (same engine)
            for blk in range(P // 32):
                vector.transpose(
                    out=T[0:32, blk * 32 : blk * 32 + 32],
                    in_=F[blk * 32 : blk * 32 + 32, 0:32],
                ).then_inc(trans_done, 1)

        @block.scalar
        def _(scalar):
            A = 1.0 / np.sqrt(float(N) * (N - 1))

            def act_accumulate(j, m, k):
                # sum (Copy+accum) then sumsq (Square+accum), in-place on xt
                scalar.wait_ge(dma_in[j], 16)
                base_as = act_stats_after[j] - 2 * m
                for r in range(m):
                    row = xt[:, k, r * N : (r + 1) * N]
                    scalar.activation(
                        out=row,
                        in_=row,
                        func=mybir.ActivationFunctionType.Copy,
                        accum_out=acc[:, r, 0:1],
                    ).then_inc(act_stats, 1)
                    # observe the Copy (xt write + acc[0]) before Square
                    scalar.wait_ge(act_stats, base_as + 2 * r + 1)
                    scalar.activation(
                        out=row,
                        in_=row,
                        func=mybir.ActivationFunctionType.Square,
                        accum_out=acc[:, r, 1:2],
                    ).then_inc(act_stats, 1)

            def stat_dsts(j, m, s):
                """Per-row (mean_dst, std_dst) APs for this slab's stats."""
                if m == 1:
                    scalar.wait_ge(trans_done, 1)  # F memset observed
                    ti = TAIL.index(j)
                    return [(F[:, 2 * ti : 2 * ti + 1], F[:, 2 * ti + 1 : 2 * ti + 2])]
                return [
                    (res[:, s, 0, r : r + 1], res[:, s, 1, r : r + 1])
                    for r in range(m)
                ]

            def finish_direct(j, m, s):
                """res RAW wait + direct out-DMA for m>=2 slabs."""
                b, c0 = divmod(base_rows[j], C)
                scalar.wait_ge(act_done, actd_after[j])
                dst = bass.AP(
                    tensor=out_ext,
                    offset=b * 2 * C + c0,
                    ap=[[m, P], [C, 2], [1, m]],
                )
                scalar.dma_start(out=dst, in_=res[:, s, :, 0:m]).then_inc(
                    dma_out[s], 16
                )

            def act_epilogue(j, m):
                s = j % NSMALL
                dsts = stat_dsts(j, m, s)
                scalar.wait_ge(act_stats, act_stats_after[j])
                if m >= 2 and wait_before[j]:
                    scalar.wait_ge(dma_out[j % NSMALL], wait_before[j])
                ad = actd_after[j] - 4 * m  # running act_done value
                for r in range(m):
                    mean_dst, std_dst = dsts[r]
                    # mean = sum / N
                    scalar.activation(
                        out=mean_dst,
                        in_=acc[:, r, 0:1],
                        func=mybir.ActivationFunctionType.Copy,
                        scale=1.0 / N,
                    ).then_inc(act_done, 1)
                    # tmp = (sum*A)^2 = sum^2/(N(N-1))
                    scalar.activation(
                        out=acc[:, r, 2:3],
                        in_=acc[:, r, 0:1],
                        func=mybir.ActivationFunctionType.Square,
                        scale=A,
                    ).then_inc(act_done, 1)
                    ad += 2
                    scalar.wait_ge(act_done, ad)
                    scalar.activation(
                        out=acc[:, r, 2:3],
                        in_=acc[:, r, 2:3],
                        func=mybir.ActivationFunctionType.Copy,
                        scale=-1.0,
                    ).then_inc(act_done, 1)
                    ad += 1
                    scalar.wait_ge(act_done, ad)
                    # std = sqrt(sumsq/(N-1) - sum^2/(N(N-1)))
                    scalar.activation(
                        out=std_dst,
                        in_=acc[:, r, 1:2],
                        func=mybir.ActivationFunctionType.Sqrt,
                        scale=1.0 / (N - 1),
                        bias=acc[:, r, 2:3],
                    ).then_inc(act_done, 1)
                    ad += 1
                if m >= 2:
                    finish_direct(j, m, s)

            def dve_epilogue(j, m):
                s = j % NSMALL
                dsts = stat_dsts(j, m, s)
                scalar.wait_ge(mv_ready, mv_after[j])
                if m >= 2 and wait_before[j]:
                    scalar.wait_ge(dma_out[j % NSMALL], wait_before[j])
                for r in range(m):
                    mean_dst, std_dst = dsts[r]
                    scalar.copy(out=mean_dst, in_=mv[:, s, r, 0:1]).then_inc(
                        act_done, 1
                    )
                    scalar.activation(
                        out=std_dst,
                        in_=mv[:, s, r, 1:2],
                        func=mybir.ActivationFunctionType.Sqrt,
                        scale=float(N) / (N - 1),
                    ).then_inc(act_done, 1)
                if m >= 2:
                    finish_direct(j, m, s)

            # Emission order: an ACT slab's accumulate is hoisted before the
            # previous slab's epilogue so it starts at DMA arrival instead of
            # queueing behind mv_ready stalls.
            emitted = set()
            for j, m in enumerate(SLABS):
                nj = j + HOIST
                if nj < nslabs and CONSUMER[nj] == "act" and nj not in emitted:
                    act_accumulate(nj, SLABS[nj], nj % NBUF)
                    emitted.add(nj)
                if CONSUMER[j] == "dve":
                    dve_epilogue(j, m)
                else:
                    if j not in emitted:
                        act_accumulate(j, m, j % NBUF)
                        emitted.add(j)
                    act_epilogue(j, m)

    return nc


def kernel(f_vol: np.ndarray) -> np.ndarray:
    from concourse.bass_utils import run_bass_kernel_spmd

    if "nc" not in _CACHE:
        _CACHE["nc"] = _build()
    nc = _CACHE["nc"]

    f_vol = np.ascontiguousarray(f_vol, dtype=np.float32)
    in_maps = [
        {"f_vol": f_vol[i * B_LOCAL : (i + 1) * B_LOCAL]} for i in range(N_CORES)
    ]
    res = run_bass_kernel_spmd(nc, in_maps, core_ids=list(range(N_CORES)))
    return np.concatenate([res.results[i]["out"] for i in range(N_CORES)], axis=0)
